# revision 26
# baseline (speedup 1.0000x reference)
"""MoE layer (E=8 experts, top-2, swiglu FFN) on 8 Trainium2 NeuronCores.

Strategy: expert dispatch on host + tensor-parallel-over-hidden on device.
  - Router (logits -> top-2 -> softmax weights) computed on host with the
    exact same jnp ops as the reference, so top-k decisions match bit-for-bit.
  - Tokens are gathered per expert on host into one flat dispatch list
    shared by all cores.
  - Every core processes ALL experts' token lists, but only a 1/8 slice of
    each expert's hidden units (h1 cols [256c:256c+256) paired with the
    matching h2 cols, and the matching W2 rows). The hidden split commutes
    with swiglu, so partial outputs sum exactly. Per-core work is exactly
    sum(n_e)/8 token-equivalents — perfectly balanced, no padding (the
    matmul moving dim takes arbitrary tile sizes).
  - On-device dataflow: features on partitions / tokens on the moving free
    dim; H^T = W1ᵀ·X^T, swiglu, Y^T = W2ᵀ·S^T — no on-chip transposes.
    bf16 matmuls, fp32 accumulate, fp16 partial-y output.
  - Host combines: out[token] += w_k * (sum_c y_c + b2[e]).

The kernel is PE-bound at ~93% busy; all recoverable time is at the start
ramp and end drain. Hardware facts this schedule is built around (measured
via NTFF traces on trn2):
  - The PE clock (HAM gate) runs 2.4GHz only after ~3.4us of sustained
    matmul activity; ANY >1us PE gap resets it to 1.2GHz for ~3us. Hence
    the warm matmuls must run gaplessly into the first real matmul, and
    cold-start DMA granules must arrive with sub-1us spacing.
  - HWDGE rings move ~141-158GB/s warm, but only ~60-100GB/s for several
    us after sitting idle (cold ramp). The early window (all queues cold,
    ~8.6-18us) is fabric-capped ~270GB/s total.
  - An engine BLOCKS on DMA-ring slots while issuing dma_start; a long
    pre-emitted load schedule must live on the sync engine (no compute
    duties). Putting it on the scalar engine head-of-line-blocks every
    silu (measured 30us PE stall).
  - Multiple granule DMAs into ONE tile share lumped waits at the
    consumer (compile-time wait coarsening) — cold-start quarters go to
    SEPARATE SBUF tiles so each k-group gates on its own semaphore.
  - Queue plan: sync HWDGE carries x plus the e1..e7 weights interleaved
    in consumption order (plus final-store halves at the very end);
    scalar HWDGE carries only ~1MB of e0 cold-start weights so its
    stream reaches tile0's silu on time; gpsimd SWDGE carries b1 + e0
    spillover quarters, then all mid-kernel y stores. The last 8 tiles
    trickle store traffic through scalar to keep its ring warm, and the
    last 4 split halves across both HW rings (a cold ring drains the
    final store at ~60GB/s, exposing ~8us after the last matmul).
"""

import numpy as np
import ml_dtypes

E = 8
K = 2
DIM = 1024
HID = 2048
H2 = 2 * HID  # fc1 output width (4096)
P = 128
KO1 = DIM // P  # 8 k-tiles for fc1
SH = HID // 8  # 256 hidden units per core-shard
SW = 2 * SH  # 512 fc1 output cols per shard (h1 half + h2 half)
MO1 = SW // P  # 4 m-tiles for fc1 shard output (0,1 = h1; 2,3 = h2)
KO2 = SH // P  # 2 k-tiles for fc2 shard
MO2 = DIM // P  # 8 m-tiles for fc2 output
TN = 512  # max token tile (matmul moving free dim)

_cache: dict = {}

# Extra kwargs splatted into run_bass_kernel_spmd (test harness sets this to
# enable NTFF tracing; empty by default so grading runs are unaffected).
TRACE_OPTS: dict = {}
LAST_RESULTS = None


def _tile_list(counts):
    """[(expert, tn), ...] covering each expert's token list in 512-chunks.

    Experts are ordered largest-tail-first so the final tile (and its exposed
    epilogue/store tail) is the smallest remainder tile.
    """
    exp_order = sorted(range(len(counts)), key=lambda e: -((counts[e] % TN) or TN))
    tiles = []
    for e in exp_order:
        ne = counts[e]
        n0 = 0
        while n0 < ne:
            tiles.append((e, min(TN, ne - n0)))
            n0 += TN
    return tiles


def _build(counts):
    """Build + compile the SPMD Bass program for per-expert token counts."""
    import concourse.mybir as mybir
    import concourse.tile as tile
    from concourse import bacc
    from contextlib import ExitStack

    dt = mybir.dt
    AF = mybir.ActivationFunctionType
    ALU = mybir.AluOpType

    tiles = _tile_list(counts)
    ntiles = len(tiles)

    nc = bacc.Bacc("TRN2", target_bir_lowering=False, debug=False, num_devices=8)

    xt = nc.dram_tensor(
        "xt", [ntiles, P, KO1 * TN], dt.bfloat16, kind="ExternalInput"
    ).ap()
    w1 = nc.dram_tensor("w1", [E, P, KO1 * SW], dt.bfloat16, kind="ExternalInput").ap()
    b1 = nc.dram_tensor("b1", [P, E * MO1], dt.float32, kind="ExternalInput").ap()
    w2 = nc.dram_tensor("w2", [E, P, KO2 * DIM], dt.bfloat16, kind="ExternalInput").ap()
    yt = nc.dram_tensor(
        "yt", [ntiles, P, MO2 * TN], dt.float16, kind="ExternalOutput"
    ).ap()

    with tile.TileContext(nc) as tc, ExitStack() as ctx:
        wpool = ctx.enter_context(tc.tile_pool(name="weights", bufs=1))
        xpool = ctx.enter_context(tc.tile_pool(name="xp", bufs=5))
        spool = ctx.enter_context(tc.tile_pool(name="sp", bufs=2))
        opool = ctx.enter_context(tc.tile_pool(name="op", bufs=3))
        tpool = ctx.enter_context(tc.tile_pool(name="tp", bufs=4))
        pspool = ctx.enter_context(tc.tile_pool(name="ps", bufs=4, space="PSUM"))
        pypool = ctx.enter_context(tc.tile_pool(name="py", bufs=4, space="PSUM"))

        w1_sb = wpool.tile([P, E, KO1, SW], dt.bfloat16)
        w2_sb = wpool.tile([P, E, KO2, DIM], dt.bfloat16)
        b1_sb = wpool.tile([P, E * MO1], dt.float32)

        # PE warmup: junk matmuls on a zeroed tile bridge the Tile preamble /
        # DMA spin-up window (~7..12.3us) so the HAM clock gate opens before
        # real work; they must run GAPLESSLY into the first real matmul — any
        # >1us PE gap resets the HAM ramp and the whole cold start runs at
        # half clock (measured: full clock only at 23.5us with gappy start).
        # 32 warms ~= +7us of coverage (8 cold ones at ~420ns until the HAM
        # opens, then ~150ns each): the first x/w granule lands +5.5..+7.7us
        # after warm start depending on per-core DMA skew, and a gap between
        # warms and real work costs a HAM reset (~3us of half clock) on
        # whichever core defines the max exec time.
        warm_sb = wpool.tile([P, 256], dt.bfloat16)
        nc.vector.memset(warm_sb[:], 0.0)
        warm_ps = pypool.tile([P, 256], dt.float32, tag="py")
        for _ in range(32):
            nc.tensor.matmul(
                warm_ps[:],
                lhsT=warm_sb[:, :P],
                rhs=warm_sb[:, :],
                start=True,
                stop=True,
            )

        exp_order = list(dict.fromkeys(e for e, _ in tiles))
        e0 = exp_order[0]

        # Cold-start granules: slices over CONSECUTIVE k are contiguous per
        # partition in both DRAM and SBUF, so k-pair granules move 2KB lines
        # (~120GB/s) vs 1KB for single-k (~80GB/s). Spread e0's weights over
        # three queues so the 2.5MB tile-0 gate flows at aggregate bandwidth:
        #   scalar HWDGE: w1[e0] k0-3 + w2[e0]  (must stay ~1MB: the scalar
        #     ENGINE blocks on DMA-ring slots while issuing and its stream
        #     must reach tile0's silu by ~16us; more weights here head-of-
        #     line-block every silu — measured 30us PE stall)
        #   gpsimd SWDGE: b1 + w1[e0] k4-7  (idle until first y store ~20us)
        #   sync HWDGE:   x tiles (+ the rest of the weights below)
        # Cold-start weights: w1[e0] in k-pair quarters, each loaded into its
        # OWN SBUF tile so every quarter gets a distinct completion semaphore
        # — with one shared tile the compile-time wait placement lumps the
        # granule sems together and the PE's k2 group ends up waiting for k67
        # (measured 3us gap + HAM reset). Quarters k01/k23/k45 ride the
        # scalar HWDGE (fastest cold ring that isn't carrying x); k67 leads
        # the gpsimd SWDGE (slow cold ring, but k67 isn't needed until ~19us).
        # All of e0's tiles read w1 through these quarter tiles.
        w1q = [wpool.tile([P, 2, SW], dt.bfloat16, name=f"w1q{j}") for j in range(4)]
        Q = 2 * SW  # one k-pair quarter (256KB)
        for j in (0, 1, 2):
            nc.scalar.dma_start(
                w1q[j][:].rearrange("p k n -> p (k n)"), w1[e0, :, j * Q : (j + 1) * Q]
            )
        nc.scalar.dma_start(w2_sb[:, e0].rearrange("p k n -> p (k n)"), w2[e0])

        def w1ap(e, k, c0, c1):
            """lhsT slice for expert e's fc1 k-tile (e0 lives in w1q)."""
            if e == e0:
                return w1q[k // 2][:, k % 2, c0:c1]
            return w1_sb[:, e, k, c0:c1]

        # sync HWDGE: x tiles + remaining expert weights, pre-emitted in
        # consumption order (the sync engine has no compute duties, so ring
        # blocking is harmless). Tile 0 loads k-pair quarters into separate
        # tiles (distinct sems, see w1q above); tiles 1-2 load halves; later
        # tiles whole (queue runs well ahead by then).
        x0q = [
            wpool.tile([P, 2, TN], dt.bfloat16, name=f"x0q{j}") for j in range(4)
        ]
        for j in range(2):
            nc.sync.dma_start(
                x0q[j][:].rearrange("p k n -> p (k n)"),
                xt[0, :, j * 2 * TN : (j + 1) * 2 * TN],
            )
        # k45 and k67 ride gpsimd (right behind w1q3): the sync ring alone
        # paces late quarters ~1-2.5us behind the PE on DMA-slow cores;
        # gpsimd is idle and w1q3 isn't needed until ~18us, so its chain has
        # slack for both.
        for j in (2, 3):
            nc.gpsimd.dma_start(
                x0q[j][:].rearrange("p k n -> p (k n)"),
                xt[0, :, j * 2 * TN : (j + 1) * 2 * TN],
            )
        # w-k67 and b1 close the gpsimd cold chain (needed ~18us / ~19us).
        nc.gpsimd.dma_start(
            w1q[3][:].rearrange("p k n -> p (k n)"), w1[e0, :, 3 * Q : 4 * Q]
        )
        nc.gpsimd.dma_start(b1_sb[:], b1[:])
        x_tiles = {0: None}

        def emit_x(ti, granules=((0, 8),)):
            if ti >= ntiles or ti in x_tiles:
                return
            xx = xpool.tile([P, KO1, TN], dt.bfloat16, tag="x", name=f"x_{ti}")
            flat = xx[:].rearrange("p k n -> p (k n)")
            for a, b in granules:
                nc.sync.dma_start(flat[:, a * TN : b * TN], xt[ti, :, a * TN : b * TN])
            x_tiles[ti] = xx

        emit_x(1, [(0, 4), (4, 8)])
        emit_x(2)
        emit_x(3)

        def xap(ti, k, tn):
            """rhs slice for tile ti's fc1 k-tile (tile 0 lives in x0q)."""
            if ti == 0:
                return x0q[k // 2][:, k % 2, :tn]
            return x_tiles[ti][:, k, :tn]

        nxt = 4
        for e in exp_order[1:]:
            nc.sync.dma_start(w1_sb[:, e].rearrange("p k n -> p (k n)"), w1[e])
            nc.sync.dma_start(w2_sb[:, e].rearrange("p k n -> p (k n)"), w2[e])
            emit_x(nxt)
            emit_x(nxt + 1)
            nxt += 2
        for ti in range(nxt, ntiles):
            emit_x(ti)

        for ti, (e, tn) in enumerate(tiles):
            # Stage 1: 4 m-tiles (2 h1 + 2 h2), k-loop outermost so each
            # arriving DMA slice unblocks dense matmuls during the cold start
            # (full-width k-granules feed both groups).
            # Tile 0 runs all 4 banks in ONE k-loop: the PE instruction order
            # is fixed at compile time, and the two-group order would leave a
            # ~2us PE gap at k=4 waiting for x0's second half (HAM down-clock
            # trigger). Steady-state tiles keep the two-group order, which
            # pipelines better across tile boundaries (4-at-once allocation
            # serializes on the previous tile's full swiglu).
            s_sb = spool.tile([P, KO2, TN], dt.bfloat16, tag="s")
            ps1s = [
                pspool.tile([P, TN], dt.float32, tag="ps", name=f"ps1_{ti}_{i}")
                for i in range(2)
            ]
            ps2s = [
                pspool.tile([P, TN], dt.float32, tag="ps", name=f"ps2_{ti}_{i}")
                for i in range(2)
            ]
            if ti == 0:
                for k in range(KO1):
                    for mi in range(2):
                        nc.tensor.matmul(
                            ps1s[mi][:, :tn],
                            lhsT=w1ap(e, k, mi * P, (mi + 1) * P),
                            rhs=xap(ti, k, tn),
                            start=(k == 0),
                            stop=(k == KO1 - 1),
                        )
                    for mi in range(2):
                        nc.tensor.matmul(
                            ps2s[mi][:, :tn],
                            lhsT=w1ap(e, k, (2 + mi) * P, (3 + mi) * P),
                            rhs=xap(ti, k, tn),
                            start=(k == 0),
                            stop=(k == KO1 - 1),
                        )
            else:
                for k in range(KO1):
                    for mi in range(2):
                        nc.tensor.matmul(
                            ps1s[mi][:, :tn],
                            lhsT=w1ap(e, k, mi * P, (mi + 1) * P),
                            rhs=xap(ti, k, tn),
                            start=(k == 0),
                            stop=(k == KO1 - 1),
                        )
                for k in range(KO1):
                    for mi in range(2):
                        nc.tensor.matmul(
                            ps2s[mi][:, :tn],
                            lhsT=w1ap(e, k, (2 + mi) * P, (3 + mi) * P),
                            rhs=xap(ti, k, tn),
                            start=(k == 0),
                            stop=(k == KO1 - 1),
                        )
            for mi in range(2):
                t1 = tpool.tile([P, TN], dt.float32, tag="t1")
                # t1 = silu(h1 + b1a)
                nc.scalar.activation(
                    t1[:, :tn],
                    ps1s[mi][:, :tn],
                    AF.Silu,
                    bias=b1_sb[:, e * MO1 + mi : e * MO1 + mi + 1],
                )
                # s = (h2 + b1b) * t1   (cast to bf16 on write)
                nc.vector.scalar_tensor_tensor(
                    s_sb[:, mi, :tn],
                    ps2s[mi][:, :tn],
                    b1_sb[:, e * MO1 + 2 + mi : e * MO1 + 3 + mi],
                    t1[:, :tn],
                    op0=ALU.add,
                    op1=ALU.mult,
                )

            # Stage 2: partial y for this shard (no b2 — host adds it once).
            o_sb = opool.tile([P, MO2, TN], dt.float16, tag="o")
            for m2 in range(MO2):
                psy = pypool.tile([P, TN], dt.float32, tag="py", name=f"psy_{ti}_{m2}")
                for k2 in range(KO2):
                    nc.tensor.matmul(
                        psy[:, :tn],
                        lhsT=w2_sb[:, e, k2, m2 * P : (m2 + 1) * P],
                        rhs=s_sb[:, k2, :tn],
                        start=(k2 == 0),
                        stop=(k2 == KO2 - 1),
                    )
                # Alternate the psum->SBUF copies between ScalarE and VectorE:
                # a single engine can't keep up at this tile rate.
                if m2 % 2 == 0:
                    nc.scalar.copy(o_sb[:, m2, :tn], psy[:, :tn])
                else:
                    nc.vector.tensor_copy(o_sb[:, m2, :tn], psy[:, :tn])
            yt_t = yt[ti].rearrange("p (m n) -> p m n", n=TN)
            if ti >= ntiles - 4:
                # Final tiles: quarter-split across the two HWDGE rings so
                # the last transfer is 0.25MB (~1.8us) and the teardown isn't
                # left waiting on a half-MB drain.
                nc.sync.dma_start(yt_t[:, 0:2, :tn], o_sb[:, 0:2, :tn])
                nc.sync.dma_start(yt_t[:, 2:4, :tn], o_sb[:, 2:4, :tn])
                nc.scalar.dma_start(yt_t[:, 4:6, :tn], o_sb[:, 4:6, :tn])
                nc.scalar.dma_start(yt_t[:, 6:8, :tn], o_sb[:, 6:8, :tn])
            elif ti >= ntiles - 8:
                # Pre-warm the scalar ring at tile cadence: a ring idle for
                # ~40us transfers the final stores at cold-ramp rate
                # (~60GB/s, measured +8.5us drain); real store traffic keeps
                # it ramped.
                nc.gpsimd.dma_start(yt_t[:, 0:4, :tn], o_sb[:, 0:4, :tn])
                nc.scalar.dma_start(yt_t[:, 4:8, :tn], o_sb[:, 4:8, :tn])
            else:
                nc.gpsimd.dma_start(yt_t[:, :, :tn], o_sb[:, :, :tn])

    nc.compile()
    return nc


def _get_nc(counts):
    key = tuple(counts)
    if key not in _cache:
        _cache[key] = _build(counts)
    return _cache[key]


def _route(x, router_w, router_b):
    """Replicate the reference router bit-for-bit (same jnp ops, same backend)."""
    import jax
    import jax.numpy as jnp

    logits = jnp.einsum("btd,ed->bte", x, router_w) + router_b
    topk_val, topk_idx = jax.lax.top_k(logits, K)
    weights = jax.nn.softmax(topk_val, axis=-1)
    return np.asarray(topk_idx), np.asarray(weights)


def kernel(x, router_w, router_b, W1, b1, W2, b2):
    from concourse.bass_utils import run_bass_kernel_spmd

    x = np.asarray(x, dtype=np.float32)
    router_w = np.asarray(router_w, dtype=np.float32)
    router_b = np.asarray(router_b, dtype=np.float32)
    W1 = np.asarray(W1, dtype=np.float32)
    b1 = np.asarray(b1, dtype=np.float32)
    W2 = np.asarray(W2, dtype=np.float32)
    b2 = np.asarray(b2, dtype=np.float32)

    B, T, _ = x.shape
    NTOK = B * T
    x_flat = x.reshape(NTOK, DIM)

    topk_idx, topk_w = _route(x, router_w, router_b)
    topk_idx = topk_idx.reshape(NTOK, K)
    topk_w = topk_w.reshape(NTOK, K).astype(np.float32)

    # Per-expert token lists + combine weights
    idx_list, w_list = [], []
    for e in range(E):
        rows, cols = np.nonzero(topk_idx == e)
        idx_list.append(rows.astype(np.int64))
        w_list.append(topk_w[rows, cols])
    counts = [len(i) for i in idx_list]

    nc = _get_nc(counts)
    tiles = _tile_list(counts)
    ntiles = len(tiles)

    bf16 = ml_dtypes.bfloat16

    # Shared token dispatch: one tile-major array used by every core.
    xt = np.zeros((ntiles, P, KO1 * TN), bf16)
    tpos = [0] * E
    for ti, (e, tn) in enumerate(tiles):
        rows = x_flat[idx_list[e][tpos[e] : tpos[e] + tn]]  # [tn, DIM]
        tpos[e] += tn
        # [j, ko*P+p] -> [p, ko*TN+j]
        blk = rows.T.reshape(KO1, P, tn).transpose(1, 0, 2)  # [P, KO1, tn]
        xt[ti].reshape(P, KO1, TN)[:, :, :tn] = blk.astype(bf16)

    in_maps = []
    for c in range(E):
        cols = np.r_[SH * c : SH * (c + 1), HID + SH * c : HID + SH * (c + 1)]
        w1c = np.zeros((E, P, KO1 * SW), bf16)
        w2c = np.zeros((E, P, KO2 * DIM), bf16)
        b1c = np.zeros((P, E * MO1), np.float32)
        for e in range(E):
            w1s = W1[e][:, cols]  # [DIM, SW]
            w1c[e] = (
                w1s.reshape(KO1, P, SW).transpose(1, 0, 2).reshape(P, KO1 * SW)
            ).astype(bf16)
            w2s = W2[e][SH * c : SH * (c + 1)]  # [SH, DIM]
            w2c[e] = (
                w2s.reshape(KO2, P, DIM).transpose(1, 0, 2).reshape(P, KO2 * DIM)
            ).astype(bf16)
            b1c[:, e * MO1 : (e + 1) * MO1] = b1[e][cols].reshape(MO1, P).T
        in_maps.append({"xt": xt, "w1": w1c, "b1": b1c, "w2": w2c})

    res = run_bass_kernel_spmd(nc, in_maps, core_ids=list(range(E)), **TRACE_OPTS)
    global LAST_RESULTS
    LAST_RESULTS = res

    # Sum the 8 shard partials, then combine per expert.
    y_sum = res.results[0]["yt"].astype(np.float32)
    for c in range(1, E):
        y_sum += res.results[c]["yt"]
    # [ti, p, m2*TN+j] -> per-tile [tn, DIM]
    y_sum = y_sum.reshape(ntiles, P, MO2, TN).transpose(0, 3, 2, 1)

    out_flat = np.zeros((NTOK, DIM), np.float32)
    tpos = [0] * E
    for ti, (e, tn) in enumerate(tiles):
        idx = idx_list[e][tpos[e] : tpos[e] + tn]
        w = w_list[e][tpos[e] : tpos[e] + tn]
        tpos[e] += tn
        y = y_sum[ti, :tn].reshape(tn, DIM) + b2[e]
        out_flat[idx] += w[:, None] * y
    return out_flat.reshape(B, T, DIM)
